# revision 7
# baseline (speedup 1.0000x reference)
"""Trainium2 Bass kernel for group-quant (fake int8, V=64) + Linear.

reference math (per row of x):
    absmax over feature-groups of 64 -> delta = max(2*absmax/254, 1e-5)
    xq = clip(round(x/delta), -127, 127) * delta      (fake quant)
    out = xq @ W.T + b

Sharding: data-parallel on tokens across 8 cores (1024 rows each);
W (pre-transposed/packed fp16 on host) + b (fp16) replicated.

Device schedule per core (v2 — W-half resident, t-tile pipeline):
  quant runs per 128-token tile in natural layout, column-split across
  the vector and gpsimd engines (group absmax reduce, exact RNE round
  via the +/-1.5*2^23 trick, dequant to fp16), then ONE whole-tile
  SBUF->SBUF XBAR transpose (scalar queue, isolated from copy DMAs)
  produces x~^T [128k, KT, 128t].  Matmuls run in two phases, each with
  half of W^T (4 oc-chunks of 512, 16.8MB fp16) resident in SBUF:
  per t-tile, k-outer/oc-inner so 4 consecutive matmuls share one
  stationary (LDWEIGHTS amortized 4x), accumulating into 4 PSUM banks,
  ping-ponged with the other 4 across t-tiles.  First/last tiles of
  each phase run oc-major to stagger W DMA arrival/swap.  x~^T tiles
  are spilled to DRAM in phase A and re-loaded in phase B.  PSUM is
  evacuated fused with the bias add on vector; output DMAs ride the
  vector queue so they never head-of-line-block W loads (sync queue).
"""

import numpy as np

import concourse.bass as bass
import concourse.mybir as mybir
import concourse.tile as tile
from concourse.bass_utils import run_bass_kernel_spmd

N_CORES = 8
MAGIC = 1.5 * 2.0**23      # fp32 round-to-nearest-even constant
QSCALE = 1.0 / 127.0       # 2/(qmax-qmin) with qmax=127, qmin=-127
DELTA_MIN = 1e-5


def _split_multiwait(nc):
    """This walrus build allows at most ONE sync wait per instruction
    ("Too many sync wait commands", CoreV3GenImpl setupSyncWait) and none
    on Drain. Tile freely attaches several waits to one instruction, so
    post-process: move excess waits onto single-wait NoOps inserted just
    before the instruction on the same engine queue (semantics identical —
    the queue stalls at the nop instead of at the instruction)."""
    nid = 0
    for fn in nc.m.functions:
        for bb in fn.blocks:
            insts = list(bb.instructions)
            out = []
            changed = False
            for inst in insts:
                si = inst.sync_info
                waits = list(si.on_wait) if si is not None and si.on_wait else []
                limit = 0 if type(inst).__name__ == "InstDrain" else 1
                if len(waits) > limit:
                    changed = True
                    keep = waits[len(waits) - limit :] if limit else []
                    for w in waits[: len(waits) - limit]:
                        nid += 1
                        out.append(
                            mybir.InstNoOp(
                                name=f"WSPLIT-{nid}",
                                engine=inst.engine,
                                bass_nofuse=True,
                                ins=[],
                                outs=[],
                                sync_info=mybir.SyncInfo(on_wait=[w], on_update=[]),
                            )
                        )
                    si.on_wait = keep
                out.append(inst)
            if changed:
                try:
                    bb.instructions = out
                except Exception:
                    bb.instructions[:] = out


def build(T=1024, K=4096, O=4096, V=64, CV=1792, wq_split=4, split=True,
          noload=False):
    f32, f16 = mybir.dt.float32, mybir.dt.float16
    P = 128
    G = K // V                 # quant groups per row (64)
    KT = K // P                # contraction tiles (32)
    NT = T // P                # token tiles per core (8)
    OC = 512                   # oc chunk (psum bank width fp32)
    NOC = O // OC              # 8
    NPH = 2                    # W halves
    OCPH = NOC // NPH          # oc chunks per phase (4)
    GV = CV // V               # vector-side quant groups
    KQ = KT // wq_split        # W load quarters

    nc = bass.Bass()
    x = nc.dram_tensor("x", [T, K], f32, kind="ExternalInput")
    wt = nc.dram_tensor("wt", [NOC, P, KT * OC], f16, kind="ExternalInput")
    bvec = nc.dram_tensor("b", [O], f16, kind="ExternalInput")
    out = nc.dram_tensor("out", [T, O], f32, kind="ExternalOutput")
    xtd = nc.dram_tensor("xtd", [NT, P, KT * P], f16)  # x~^T spill

    mult = mybir.AluOpType.mult
    add = mybir.AluOpType.add
    sub = mybir.AluOpType.subtract
    amax_op = mybir.AluOpType.max

    with tile.TileContext(nc) as tc:
        with (
            tc.tile_pool(name="x", bufs=2) as pool_x,
            tc.tile_pool(name="xh", bufs=2) as pool_xh,
            tc.tile_pool(name="st", bufs=2) as pool_s,
            tc.tile_pool(name="xt", bufs=2) as pool_xt,
            tc.tile_pool(name="w", bufs=1) as pool_w,
            tc.tile_pool(name="bias", bufs=1) as pool_b,
            tc.tile_pool(name="o", bufs=3) as pool_o,
            tc.tile_pool(name="ps", bufs=1, space="PSUM") as pool_ps,
        ):
            def post_bias(ph):
                bt = pool_b.tile([P, OCPH, OC], f16, tag="b", name=f"b{ph}")
                bsl = bvec[ph * OCPH * OC : (ph + 1) * OCPH * OC]
                bb = bass.AP(
                    tensor=bsl.tensor, offset=bsl.offset, ap=[[0, P], *bsl.ap]
                )
                nc.sync.dma_start(out=bt.rearrange("p c o -> p (c o)"), in_=bb)
                return bt

            def post_w(ph, oc):
                wtile = pool_w.tile(
                    [P, KT, OC], f16, tag=f"w{oc}", name=f"w{ph}_{oc}"
                )
                g = ph * OCPH + oc
                for q in range(wq_split):
                    nc.sync.dma_start(
                        out=wtile[:, q * KQ : (q + 1) * KQ, :].rearrange(
                            "p k o -> p (k o)"
                        ),
                        in_=wt[g][:, q * KQ * OC : (q + 1) * KQ * OC],
                    )
                return wtile

            def emit_quant(t):
                xt_ = pool_x.tile([P, K], f32, tag="x", name=f"x{t}")
                nc.gpsimd.dma_start(out=xt_[:], in_=x[t * P : (t + 1) * P, :])
                xr = xt_.rearrange("p (g v) -> p g v", v=V)
                # gpsimd can only reduce along partitions -> reduce on vector
                amax = pool_s.tile([P, G], f32, tag="amax", name=f"amax{t}")
                nc.vector.tensor_reduce(
                    out=amax[:], in_=xr, axis=mybir.AxisListType.X,
                    op=amax_op, apply_absolute_value=True,
                )
                delta = pool_s.tile([P, G], f32, tag="delta", name=f"delta{t}")
                nc.vector.tensor_scalar(
                    out=delta[:], in0=amax[:],
                    scalar1=QSCALE, scalar2=DELTA_MIN, op0=mult, op1=amax_op,
                )
                recip = pool_s.tile([P, G], f32, tag="recip", name=f"recip{t}")
                nc.vector.reciprocal(out=recip[:], in_=delta[:])
                # x / delta  (broadcast recip over each group of V)
                nc.vector.tensor_tensor(
                    out=xr[:, :GV, :], in0=xr[:, :GV, :],
                    in1=recip[:, :GV, None].to_broadcast((P, GV, V)), op=mult,
                )
                nc.gpsimd.tensor_tensor(
                    out=xr[:, GV:, :], in0=xr[:, GV:, :],
                    in1=recip[:, GV:, None].to_broadcast((P, G - GV, V)), op=mult,
                )
                # exact fp32 round-to-nearest-even; |x/delta| <= 127 < 2^22
                nc.vector.tensor_scalar(
                    out=xt_[:, :CV], in0=xt_[:, :CV],
                    scalar1=MAGIC, scalar2=MAGIC, op0=add, op1=sub,
                )
                nc.gpsimd.tensor_scalar(
                    out=xt_[:, CV:], in0=xt_[:, CV:],
                    scalar1=MAGIC, scalar2=MAGIC, op0=add, op1=sub,
                )
                # dequant, cast to fp16 (integers <=127 exact in fp16)
                xh_t = pool_xh.tile([P, K], f16, tag="xh", name=f"xh{t}")
                xhr = xh_t.rearrange("p (g v) -> p g v", v=V)
                nc.vector.tensor_tensor(
                    out=xhr[:, :GV, :], in0=xr[:, :GV, :],
                    in1=delta[:, :GV, None].to_broadcast((P, GV, V)), op=mult,
                )
                nc.gpsimd.tensor_tensor(
                    out=xhr[:, GV:, :], in0=xr[:, GV:, :],
                    in1=delta[:, GV:, None].to_broadcast((P, G - GV, V)), op=mult,
                )
                # whole-tile XBAR transpose -> [128k, KT, 128t], then spill
                xts_t = pool_xt.tile([P, KT, P], f16, tag="xt", name=f"xts{t}")
                nc.scalar.dma_start_transpose(xts_t[:], xh_t[:])
                nc.sync.dma_start(
                    out=xtd[t], in_=xts_t.rearrange("p k q -> p (k q)")
                )
                return xts_t

            def evac(ph, t, oc, ps, bt):
                ot = pool_o.tile([P, OC], f32, tag="o", name=f"ot{ph}_{t}_{oc}")
                nc.vector.tensor_tensor(out=ot[:], in0=ps[:], in1=bt[:, oc, :], op=add)
                g = ph * OCPH + oc
                nc.gpsimd.dma_start(
                    out=out[t * P : (t + 1) * P, g * OC : (g + 1) * OC], in_=ot[:]
                )

            def mm(ps, lhsT, rhs, start, stop, first):
                inst = nc.tensor.matmul(ps, lhsT, rhs, start=start, stop=stop)
                if noload and not first:
                    try:
                        inst.ldweights = False
                    except Exception:
                        try:
                            inst.ins.ldweights = False
                        except Exception:
                            pass

            def emit_mm(ph, t, xts_t, bt, oc_major, after_group=None):
                if oc_major:
                    for oc in range(OCPH):
                        ps = pool_ps.tile([P, OC], f32, tag=f"ps{t % 2}_{oc}", name=f"ps{ph}_{t}_{oc}")
                        for kt in range(KT):
                            mm(ps[:], xts_t[:, kt, :], wcur[oc][:, kt, :],
                               kt == 0, kt == KT - 1, True)
                        evac(ph, t, oc, ps, bt)
                        if after_group is not None:
                            after_group(oc)
                else:
                    pss = [
                        pool_ps.tile([P, OC], f32, tag=f"ps{t % 2}_{oc}",
                                     name=f"ps{ph}_{t}_{oc}")
                        for oc in range(OCPH)
                    ]
                    for kt in range(KT):
                        for oc in range(OCPH):
                            mm(pss[oc][:], xts_t[:, kt, :], wcur[oc][:, kt, :],
                               kt == 0, kt == KT - 1, oc == 0)
                    for oc in range(OCPH):
                        evac(ph, t, oc, pss[oc], bt)

            # ---- phase A (oc 0..3 resident), quant interleaved ----
            btA = post_bias(0)
            wcur = [post_w(0, oc) for oc in range(OCPH)]
            tiles = {}
            for i in range(NT + 1):
                if i < NT:
                    tiles[i] = emit_quant(i)
                if i >= 1:
                    t = i - 1
                    if t == NT - 1:
                        def swapcb(oc):
                            wcur[oc] = post_w(1, oc)
                        emit_mm(0, t, tiles[t], btA, True, swapcb)
                    else:
                        emit_mm(0, t, tiles[t], btA, t == 0)

            # ---- phase B (oc 4..7 resident), x~^T re-loaded from DRAM ----
            def reload(t):
                xr_t = pool_xt.tile([P, KT, P], f16, tag="xt", name=f"xtr{t}")
                nc.sync.dma_start(
                    out=xr_t.rearrange("p k q -> p (k q)"), in_=xtd[t]
                )
                return xr_t

            rel = {0: reload(0), 1: reload(1)}
            btB = post_bias(1)
            for t in range(NT):
                if t + 2 < NT:
                    rel[t + 2] = reload(t + 2)
                emit_mm(1, t, rel[t], btB, t == 0)

    if split:
        _split_multiwait(nc)
    return nc


_CACHED = {}

# test-harness knobs (kernel() defaults are what the grader uses)
TRACE = False
LAST_RESULT = None
BUILD_KW = {}


def _get_nc(shape_key):
    if shape_key not in _CACHED:
        T, K, O = shape_key
        _CACHED[shape_key] = build(T=T, K=K, O=O, **BUILD_KW)
    return _CACHED[shape_key]


def pack_w(W: np.ndarray, OC: int = 512, P: int = 128) -> np.ndarray:
    # [out,in] -> W^T [in,out] fp16, packed [NOC, P, KT*OC] so each per-core
    # o-chunk W load is one fully contiguous DMA
    K, O = W.shape[1], W.shape[0]
    KT, NOC = K // P, O // OC
    wt = np.ascontiguousarray(W.T).astype(np.float16)         # [K, O]
    z = wt.reshape(KT, P, NOC, OC).transpose(2, 1, 0, 3)      # [NOC, P, KT, OC]
    return np.ascontiguousarray(z.reshape(NOC, P, KT * OC))


def kernel(x: np.ndarray, W: np.ndarray, b: np.ndarray) -> np.ndarray:
    global LAST_RESULT
    n, k = x.shape               # 8192, 4096
    o = W.shape[0]               # 4096
    assert n % N_CORES == 0
    tpc = n // N_CORES
    nc = _get_nc((tpc, k, o))

    wt = pack_w(W)
    b16 = np.ascontiguousarray(b.astype(np.float16))
    xs = np.ascontiguousarray(x.astype(np.float32)).reshape(N_CORES, tpc, k)
    in_maps = [{"x": xs[i], "wt": wt, "b": b16} for i in range(N_CORES)]
    res = run_bass_kernel_spmd(nc, in_maps, list(range(N_CORES)), trace=TRACE)
    LAST_RESULT = res
    return np.concatenate([res.results[i]["out"] for i in range(N_CORES)], axis=0)


# revision 9
# speedup vs baseline: 1.3376x; 1.3376x over previous
"""Trainium2 Bass kernel for group-quant (fake int8, V=64) + Linear.

reference math (per row of x):
    absmax over feature-groups of 64 -> delta = max(2*absmax/254, 1e-5)
    xq = clip(round(x/delta), -127, 127) * delta      (fake quant)
    out = xq @ W.T + b

Sharding: data-parallel on tokens across 8 cores (1024 rows each);
W (pre-transposed/packed fp16 on host) + b (fp16) replicated.

Device schedule per core (v2 — W-half resident, t-tile pipeline):
  quant runs per 128-token tile in natural layout, column-split across
  the vector and gpsimd engines (group absmax reduce, exact RNE round
  via the +/-1.5*2^23 trick, dequant to fp16), then ONE whole-tile
  SBUF->SBUF XBAR transpose (scalar queue, isolated from copy DMAs)
  produces x~^T [128k, KT, 128t].  Matmuls run in two phases, each with
  half of W^T (4 oc-chunks of 512, 16.8MB fp16) resident in SBUF:
  per t-tile, k-outer/oc-inner so 4 consecutive matmuls share one
  stationary (LDWEIGHTS amortized 4x), accumulating into 4 PSUM banks,
  ping-ponged with the other 4 across t-tiles.  First/last tiles of
  each phase run oc-major to stagger W DMA arrival/swap.  x~^T tiles
  are spilled to DRAM in phase A and re-loaded in phase B.  PSUM is
  evacuated fused with the bias add on vector; output DMAs ride the
  vector queue so they never head-of-line-block W loads (sync queue).
"""

import numpy as np

import concourse.bass as bass
import concourse.mybir as mybir
import concourse.tile as tile
from concourse.bass_utils import run_bass_kernel_spmd

N_CORES = 8
MAGIC = 1.5 * 2.0**23      # fp32 round-to-nearest-even constant
QSCALE = 1.0 / 127.0       # 2/(qmax-qmin) with qmax=127, qmin=-127
DELTA_MIN = 1e-5


def _split_multiwait(nc):
    """This walrus build allows at most ONE sync wait per instruction
    ("Too many sync wait commands", CoreV3GenImpl setupSyncWait) and none
    on Drain. Tile freely attaches several waits to one instruction, so
    post-process: move excess waits onto single-wait NoOps inserted just
    before the instruction on the same engine queue (semantics identical —
    the queue stalls at the nop instead of at the instruction)."""
    nid = 0
    for fn in nc.m.functions:
        for bb in fn.blocks:
            insts = list(bb.instructions)
            out = []
            changed = False
            for inst in insts:
                si = inst.sync_info
                waits = list(si.on_wait) if si is not None and si.on_wait else []
                limit = 0 if type(inst).__name__ == "InstDrain" else 1
                if len(waits) > limit:
                    changed = True
                    keep = waits[len(waits) - limit :] if limit else []
                    for w in waits[: len(waits) - limit]:
                        nid += 1
                        out.append(
                            mybir.InstNoOp(
                                name=f"WSPLIT-{nid}",
                                engine=inst.engine,
                                bass_nofuse=True,
                                ins=[],
                                outs=[],
                                sync_info=mybir.SyncInfo(on_wait=[w], on_update=[]),
                            )
                        )
                    si.on_wait = keep
                out.append(inst)
            if changed:
                try:
                    bb.instructions = out
                except Exception:
                    bb.instructions[:] = out


def build(T=1024, K=4096, O=4096, V=64, CV=1280, wq_split=4, split=True,
          noload=False):
    f32, f16 = mybir.dt.float32, mybir.dt.float16
    P = 128
    G = K // V                 # quant groups per row (64)
    KT = K // P                # contraction tiles (32)
    NT = T // P                # token tiles per core (8)
    OC = 512                   # oc chunk (psum bank width fp32)
    NOC = O // OC              # 8
    NPH = 2                    # W halves
    OCPH = NOC // NPH          # oc chunks per phase (4)
    GV = CV // V               # vector-side quant groups
    KQ = KT // wq_split        # W load quarters

    nc = bass.Bass()
    x = nc.dram_tensor("x", [T, K], f32, kind="ExternalInput")
    wt = nc.dram_tensor("wt", [NOC, P, KT * OC], f16, kind="ExternalInput")
    bvec = nc.dram_tensor("b", [O], f16, kind="ExternalInput")
    out = nc.dram_tensor("out", [T, O], f32, kind="ExternalOutput")
    xtd = nc.dram_tensor("xtd", [NT, P, KT * P], f16)  # x~^T spill

    mult = mybir.AluOpType.mult
    add = mybir.AluOpType.add
    sub = mybir.AluOpType.subtract
    amax_op = mybir.AluOpType.max

    with tile.TileContext(nc) as tc:
        with (
            tc.tile_pool(name="x", bufs=2) as pool_x,
            tc.tile_pool(name="xh", bufs=2) as pool_xh,
            tc.tile_pool(name="st", bufs=2) as pool_s,
            tc.tile_pool(name="xt", bufs=2) as pool_xt,
            tc.tile_pool(name="w", bufs=1) as pool_w,
            tc.tile_pool(name="bias", bufs=1) as pool_b,
            tc.tile_pool(name="o", bufs=3) as pool_o,
            tc.tile_pool(name="ps", bufs=1, space="PSUM") as pool_ps,
        ):
            def post_bias(ph):
                bt = pool_b.tile([P, OCPH, OC], f16, tag="b", name=f"b{ph}")
                bsl = bvec[ph * OCPH * OC : (ph + 1) * OCPH * OC]
                bb = bass.AP(
                    tensor=bsl.tensor, offset=bsl.offset, ap=[[0, P], *bsl.ap]
                )
                nc.sync.dma_start(out=bt.rearrange("p c o -> p (c o)"), in_=bb)
                return bt

            def post_w(ph, oc):
                wtile = pool_w.tile(
                    [P, KT, OC], f16, tag=f"w{oc}", name=f"w{ph}_{oc}"
                )
                g = ph * OCPH + oc
                for q in range(wq_split):
                    nc.sync.dma_start(
                        out=wtile[:, q * KQ : (q + 1) * KQ, :].rearrange(
                            "p k o -> p (k o)"
                        ),
                        in_=wt[g][:, q * KQ * OC : (q + 1) * KQ * OC],
                    )
                return wtile

            def emit_quant(t):
                xt_ = pool_x.tile([P, K], f32, tag="x", name=f"x{t}")
                nc.gpsimd.dma_start(out=xt_[:], in_=x[t * P : (t + 1) * P, :])
                xr = xt_.rearrange("p (g v) -> p g v", v=V)
                # gpsimd can only reduce along partitions -> reduce on vector
                amax = pool_s.tile([P, G], f32, tag="amax", name=f"amax{t}")
                nc.vector.tensor_reduce(
                    out=amax[:], in_=xr, axis=mybir.AxisListType.X,
                    op=amax_op, apply_absolute_value=True,
                )
                delta = pool_s.tile([P, G], f32, tag="delta", name=f"delta{t}")
                nc.vector.tensor_scalar(
                    out=delta[:], in0=amax[:],
                    scalar1=QSCALE, scalar2=DELTA_MIN, op0=mult, op1=amax_op,
                )
                recip = pool_s.tile([P, G], f32, tag="recip", name=f"recip{t}")
                nc.vector.reciprocal(out=recip[:], in_=delta[:])
                # x / delta  (broadcast recip over each group of V)
                nc.vector.tensor_tensor(
                    out=xr[:, :GV, :], in0=xr[:, :GV, :],
                    in1=recip[:, :GV, None].to_broadcast((P, GV, V)), op=mult,
                )
                nc.gpsimd.tensor_tensor(
                    out=xr[:, GV:, :], in0=xr[:, GV:, :],
                    in1=recip[:, GV:, None].to_broadcast((P, G - GV, V)), op=mult,
                )
                # exact fp32 round-to-nearest-even; |x/delta| <= 127 < 2^22
                # (full width on vector: Pool's fused 2-op tensor_scalar is
                # ~15 ns/elem vs DVE's ~0.6)
                nc.vector.tensor_scalar(
                    out=xt_[:], in0=xt_[:],
                    scalar1=MAGIC, scalar2=MAGIC, op0=add, op1=sub,
                )
                # dequant, cast to fp16 (integers <=127 exact in fp16)
                xh_t = pool_xh.tile([P, K], f16, tag="xh", name=f"xh{t}")
                xhr = xh_t.rearrange("p (g v) -> p g v", v=V)
                nc.vector.tensor_tensor(
                    out=xhr[:, :GV, :], in0=xr[:, :GV, :],
                    in1=delta[:, :GV, None].to_broadcast((P, GV, V)), op=mult,
                )
                nc.gpsimd.tensor_tensor(
                    out=xhr[:, GV:, :], in0=xr[:, GV:, :],
                    in1=delta[:, GV:, None].to_broadcast((P, G - GV, V)), op=mult,
                )
                # whole-tile XBAR transpose -> [128k, KT, 128t], then spill
                xts_t = pool_xt.tile([P, KT, P], f16, tag="xt", name=f"xts{t}")
                nc.scalar.dma_start_transpose(xts_t[:], xh_t[:])
                nc.sync.dma_start(
                    out=xtd[t], in_=xts_t.rearrange("p k q -> p (k q)")
                )
                return xts_t

            def evac(ph, t, oc, ps, bt):
                ot = pool_o.tile([P, OC], f32, tag="o", name=f"ot{ph}_{t}_{oc}")
                nc.vector.tensor_tensor(out=ot[:], in0=ps[:], in1=bt[:, oc, :], op=add)
                g = ph * OCPH + oc
                nc.gpsimd.dma_start(
                    out=out[t * P : (t + 1) * P, g * OC : (g + 1) * OC], in_=ot[:]
                )

            def mm(ps, lhsT, rhs, start, stop, first):
                inst = nc.tensor.matmul(ps, lhsT, rhs, start=start, stop=stop)
                if noload and not first:
                    try:
                        inst.ldweights = False
                    except Exception:
                        try:
                            inst.ins.ldweights = False
                        except Exception:
                            pass

            def emit_mm(ph, t, xts_t, bt, oc_major, after_group=None):
                if oc_major:
                    for oc in range(OCPH):
                        ps = pool_ps.tile([P, OC], f32, tag=f"ps{t % 2}_{oc}", name=f"ps{ph}_{t}_{oc}")
                        for kt in range(KT):
                            mm(ps[:], xts_t[:, kt, :], wcur[oc][:, kt, :],
                               kt == 0, kt == KT - 1, True)
                        evac(ph, t, oc, ps, bt)
                        if after_group is not None:
                            after_group(oc)
                else:
                    pss = [
                        pool_ps.tile([P, OC], f32, tag=f"ps{t % 2}_{oc}",
                                     name=f"ps{ph}_{t}_{oc}")
                        for oc in range(OCPH)
                    ]
                    for kt in range(KT):
                        for oc in range(OCPH):
                            mm(pss[oc][:], xts_t[:, kt, :], wcur[oc][:, kt, :],
                               kt == 0, kt == KT - 1, oc == 0)
                    for oc in range(OCPH):
                        evac(ph, t, oc, pss[oc], bt)

            # ---- phase A (oc 0..3 resident), quant interleaved ----
            btA = post_bias(0)
            wcur = [post_w(0, oc) for oc in range(OCPH)]
            tiles = {}
            for i in range(NT + 1):
                if i < NT:
                    tiles[i] = emit_quant(i)
                if i >= 1:
                    t = i - 1
                    if t == NT - 1:
                        def swapcb(oc):
                            wcur[oc] = post_w(1, oc)
                        emit_mm(0, t, tiles[t], btA, True, swapcb)
                    else:
                        emit_mm(0, t, tiles[t], btA, t == 0)

            # ---- phase B (oc 4..7 resident), x~^T re-loaded from DRAM ----
            def reload(t):
                xr_t = pool_xt.tile([P, KT, P], f16, tag="xt", name=f"xtr{t}")
                nc.sync.dma_start(
                    out=xr_t.rearrange("p k q -> p (k q)"), in_=xtd[t]
                )
                return xr_t

            rel = {0: reload(0), 1: reload(1)}
            btB = post_bias(1)
            for t in range(NT):
                if t + 2 < NT:
                    rel[t + 2] = reload(t + 2)
                emit_mm(1, t, rel[t], btB, t == 0)

    if split:
        _split_multiwait(nc)
    return nc


_CACHED = {}

# test-harness knobs (kernel() defaults are what the grader uses)
TRACE = False
LAST_RESULT = None
BUILD_KW = {}


def _get_nc(shape_key):
    if shape_key not in _CACHED:
        T, K, O = shape_key
        _CACHED[shape_key] = build(T=T, K=K, O=O, **BUILD_KW)
    return _CACHED[shape_key]


def pack_w(W: np.ndarray, OC: int = 512, P: int = 128) -> np.ndarray:
    # [out,in] -> W^T [in,out] fp16, packed [NOC, P, KT*OC] so each per-core
    # o-chunk W load is one fully contiguous DMA
    K, O = W.shape[1], W.shape[0]
    KT, NOC = K // P, O // OC
    wt = np.ascontiguousarray(W.T).astype(np.float16)         # [K, O]
    z = wt.reshape(KT, P, NOC, OC).transpose(2, 1, 0, 3)      # [NOC, P, KT, OC]
    return np.ascontiguousarray(z.reshape(NOC, P, KT * OC))


def kernel(x: np.ndarray, W: np.ndarray, b: np.ndarray) -> np.ndarray:
    global LAST_RESULT
    n, k = x.shape               # 8192, 4096
    o = W.shape[0]               # 4096
    assert n % N_CORES == 0
    tpc = n // N_CORES
    nc = _get_nc((tpc, k, o))

    wt = pack_w(W)
    b16 = np.ascontiguousarray(b.astype(np.float16))
    xs = np.ascontiguousarray(x.astype(np.float32)).reshape(N_CORES, tpc, k)
    in_maps = [{"x": xs[i], "wt": wt, "b": b16} for i in range(N_CORES)]
    res = run_bass_kernel_spmd(nc, in_maps, list(range(N_CORES)), trace=TRACE)
    LAST_RESULT = res
    return np.concatenate([res.results[i]["out"] for i in range(N_CORES)], axis=0)
